# revision 9
# baseline (speedup 1.0000x reference)
"""Trainium2 Bass kernel for factorized space-time attention.

Computation (per batch b of 8, one NeuronCore each):
  qkv = x @ w_qkv.T                      (3136, 2304)
  heads 0-5:  spatial attention over 196 patches within each of 16 frames
  heads 6-11: temporal attention over groups of 16 consecutive tokens
  out = concat(head outputs) @ w_proj.T + b_proj

Strategy (data-parallel over batch, 8 cores):
  - bf16 matmul inputs (1 cycle/row on PE vs 4 for fp32), fp32 PSUM accum.
  - software-pipelined emission: attention of superblock s is interleaved
    instruction-by-instruction with the QKV/V projection of superblock s+1
    and the output projection of s-1, so the in-order PE queue always has
    dense matmul work while attention chains (exp -> AV -> recip ->
    broadcast -> mul) wait on ACT/DVE.
  - score/exp tiles batch SAME-PARITY head pairs ((h0,h2), (h1,h3), ...)
    per frame/window. HW constraint found empirically: matmuls writing the
    same PSUM bank must share the stationary partition quadrant
    (tile_position row); mixed-quadrant writers drain concurrently from
    independent PE sub-arrays and collide fatally in the bank.
  - softmax denominators come for free as row 64 of the AV matmul via a
    ones-column appended to V; 1/denom is broadcast across partitions with
    a [1,64]-ones stationary matmul, staged to SBUF (one PSUM operand max
    per DVE op), then two DVE multiplies write attnT rows 0-63 / 64-127
    directly (DVE partition offsets are legal at 32-granularity - no
    SBUF->SBUF shift DMAs).
  - temporal block-diagonal mask multiply runs on the otherwise-idle
    GpSimd engine (base tensor op; the extended partition_broadcast DKL
    instruction measured ~10x slower than modeled on HW - avoided).
"""

import sys

if "/opt/trn_rl_repo" not in sys.path:
    sys.path.append("/opt/trn_rl_repo")

import numpy as np

import concourse.bass as bass  # noqa: F401
import concourse.mybir as mybir
import concourse.tile as tile
from concourse import bacc
from concourse.bass_utils import run_bass_kernel_spmd

F32 = mybir.dt.float32
BF16 = mybir.dt.bfloat16
AF = mybir.ActivationFunctionType

# problem dims
B = 8
F = 16
P = 196
D = 768
NH = 12
HD = 64
N = F * P  # 3136
E3 = 3 * D  # 2304
SB = 784  # superblock = lcm(196, 16) tokens
NSB = N // SB  # 4
FPSB = SB // P  # 4 frames per superblock
WPSB = SB // 112  # 7 temporal windows per superblock
SCALE = HD ** -0.5

COMPUTE = "bf16"

_CACHE = {}


def _interleave(a, b):
    """Proportionally interleave two lists of thunks."""
    out = []
    la, lb = len(a), len(b)
    if la == 0:
        return list(b)
    if lb == 0:
        return list(a)
    ia = ib = 0
    tot = la + lb
    for k in range(tot):
        # emit from whichever stream is behind its proportional pace
        if ia * lb <= ib * la and ia < la:
            out.append(a[ia]); ia += 1
        elif ib < lb:
            out.append(b[ib]); ib += 1
        else:
            out.append(a[ia]); ia += 1
    return out


def _build(compute: str, reps: int = 1):
    cdt = BF16 if compute == "bf16" else F32

    nc = bacc.Bacc("TRN2", target_bir_lowering=False, debug=False, num_devices=B)

    xt_d = nc.dram_tensor("xt", (D, N), cdt, kind="ExternalInput")
    wqkv_d = nc.dram_tensor("wqkvT", (D, E3), cdt, kind="ExternalInput")
    wproj_d = nc.dram_tensor("wprojT", (D, D), cdt, kind="ExternalInput")
    bias_d = nc.dram_tensor("bias", (D, 1), F32, kind="ExternalInput")
    mask_d = nc.dram_tensor("mask", (112, 112), cdt, kind="ExternalInput")
    out_d = nc.dram_tensor("outT", (D, N), F32, kind="ExternalOutput")

    with tile.TileContext(nc) as tc:
        with (
            tc.tile_pool(name="const", bufs=1) as cpool,
            tc.tile_pool(name="work", bufs=1) as wpool,
            tc.tile_pool(name="small", bufs=4) as spool,
            tc.tile_pool(name="psum", bufs=2, space="PSUM") as ppool,
        ):
            # ---- constants -------------------------------------------------
            wq = []
            for dc in range(6):
                t = cpool.tile([128, E3], cdt, tag=f"wq{dc}", name=f"wq{dc}")
                nc.sync.dma_start(t[:], wqkv_d.ap()[128 * dc : 128 * (dc + 1), :])
                wq.append(t)
            wp = []
            for dc in range(6):
                t = cpool.tile([128, D], cdt, tag=f"wp{dc}", name=f"wp{dc}")
                nc.sync.dma_start(t[:], wproj_d.ap()[128 * dc : 128 * (dc + 1), :])
                wp.append(t)
            bias_t = cpool.tile([128, 6], F32, tag="bias", name="bias_t")
            nc.sync.dma_start(
                bias_t[:], bias_d.ap().rearrange("(e p) one -> p (e one)", p=128)
            )
            mask2_t = cpool.tile([112, 224], cdt, tag="mask", name="mask2_t")
            nc.sync.dma_start(mask2_t[:, 0:112], mask_d.ap())
            nc.sync.dma_start(mask2_t[:, 112:224], mask_d.ap())
            zeros_col = cpool.tile([128, 1], F32, tag="zeros_c", name="zeros_col")
            nc.gpsimd.memset(zeros_col[:], 0.0)
            # stationary [1, 64] of ones for the 1/sum partition-broadcast
            ones_bc = cpool.tile([1, 64], cdt, tag="ones_bc", name="ones_bc")
            nc.gpsimd.memset(ones_bc[:], 1.0)

            # ---- per-superblock state (allocated lazily inside quanta) ----
            st = [dict() for _ in range(NSB)]

            def load_x(s):
                so = SB * s
                xts = []
                for dc in range(6):
                    t = wpool.tile([128, SB], cdt, tag=f"xts{dc}", bufs=2,
                                   name=f"xts{dc}_{s}")
                    nc.sync.dma_start(
                        t[:], xt_d.ap()[128 * dc : 128 * (dc + 1), so : so + SB]
                    )
                    xts.append(t)
                st[s]["xts"] = xts
                st[s]["qkvt"] = [None] * 12
                st[s]["vs"] = [None] * (2 * FPSB)
                st[s]["vt"] = [None] * WPSB
                st[s]["attnT"] = [
                    wpool.tile([128, SB], cdt, tag=f"attnT{i}", bufs=2,
                               name=f"attnT{i}_{s}")
                    for i in range(6)
                ]

            def qk_group(s, ti, j):
                def emit():
                    xts = st[s]["xts"]
                    if st[s]["qkvt"][ti] is None:
                        st[s]["qkvt"][ti] = wpool.tile(
                            [128, SB], cdt, tag=f"qkvt{ti}", bufs=2,
                            name=f"qkvt{ti}_{s}")
                    qt = st[s]["qkvt"][ti]
                    ps = ppool.tile([128, 392], F32, tag="mm", bufs=2,
                                    name=f"ps_qk{s}_{ti}_{j}")
                    for dc in range(6):
                        nc.tensor.matmul(
                            ps[:],
                            wq[dc][:, 128 * ti : 128 * (ti + 1)],
                            xts[dc][:, 392 * j : 392 * (j + 1)],
                            start=(dc == 0),
                            stop=(dc == 5),
                        )
                    # balance PSUM->SBUF evacuation across DVE / ACT
                    if (ti + j) % 2 == 0:
                        nc.vector.tensor_copy(qt[:, 392 * j : 392 * (j + 1)], ps[:])
                    else:
                        nc.scalar.copy(qt[:, 392 * j : 392 * (j + 1)], ps[:])
                return emit

            def v_group(s, kind, idx):
                # kind "s": spatial per-frame chunks (idx = 2*f + ci)
                # kind "t": temporal 112-token windows (idx = w)
                def emit():
                    xts = st[s]["xts"]
                    if kind == "s":
                        f, ci = idx // 2, idx % 2
                        m0, msz = (0, 128) if ci == 0 else (128, 68)
                        tok0, wcol0 = 196 * f + m0, 1536
                        vname = f"vs{f}_{ci}_{s}"
                    else:
                        msz, tok0, wcol0 = 112, 112 * idx, 1920
                        vname = f"vt{idx}_{s}"
                    vt_ = wpool.tile([msz, 390], cdt, tag=f"v{kind}{idx}", bufs=2,
                                     name=vname)
                    ps = ppool.tile([msz, 384], F32, tag="mm", bufs=2,
                                    name=f"ps_v{kind}{s}_{idx}")
                    for dc in range(6):
                        nc.tensor.matmul(
                            ps[:],
                            xts[dc][:, tok0 : tok0 + msz],
                            wq[dc][:, wcol0 : wcol0 + 384],
                            start=(dc == 0),
                            stop=(dc == 5),
                        )
                    nc.scalar.copy(
                        vt_.rearrange("p (h c) -> p h c", c=65)[:, :, 0:64],
                        ps.rearrange("p (h c) -> p h c", c=64),
                    )
                    nc.gpsimd.memset(
                        vt_.rearrange("p (h c) -> p h c", c=65)[:, :, 64:65], 1.0
                    )
                    if kind == "s":
                        st[s]["vs"][idx] = vt_
                    else:
                        st[s]["vt"][idx] = vt_
                return emit

            # HW rule (found empirically): matmuls writing the same PSUM bank
            # must share the stationary partition quadrant (tile_position
            # row). Mixed-quadrant writers drain concurrently from
            # independent PE sub-arrays and collide fatally. So score tiles
            # batch SAME-PARITY heads: (h0,h2), (h1,h3), and singles.

            # ---- spatial scoring group (frame f, heads hs, parity rows) ----
            def sp_score(s, f, hs, ex):
                # hs: list of same-parity global spatial heads (1 or 2)
                fo = 196 * f
                nh = len(hs)
                es = ex["e"].setdefault(f, {})

                def s1():
                    for ci, (m0, msz) in enumerate(((0, 128), (128, 68))):
                        ps_st = ppool.tile([msz, 196 * nh], F32, tag="st", bufs=2,
                                           name=f"ps_st{s}_{f}_{hs[0]}_{ci}")
                        for idx, h in enumerate(hs):
                            pb = 64 * (h % 2)
                            qtile = st[s]["qkvt"][h // 2]
                            ktile = st[s]["qkvt"][6 + h // 2]
                            nc.tensor.matmul(
                                ps_st[:, 196 * idx : 196 * (idx + 1)],
                                ktile[pb : pb + 64, fo + m0 : fo + m0 + msz],
                                qtile[pb : pb + 64, fo : fo + 196],
                                start=True,
                                stop=True,
                            )
                        e = spool.tile([msz, 196 * nh], cdt, tag="e", bufs=16,
                                       name=f"e{s}_{f}_{hs[0]}_{ci}")
                        nc.scalar.activation(
                            e[:], ps_st[:], AF.Exp, bias=0.0, scale=SCALE,
                        )
                        for idx, h in enumerate(hs):
                            es.setdefault(h, [None, None])
                            es[h][ci] = (e, 196 * idx)
                return s1

            # ---- spatial normalize unit (frame f, attnT pair hp) -----------
            def sp_norm(s, f, hp, ex):
                fo = 196 * f

                def s2():
                    es = ex["e"][f]
                    ps_avs = []
                    for hi in range(2):
                        h = 2 * hp + hi
                        ps_av = ppool.tile([65, 196], F32, tag="av", bufs=4,
                                           name=f"ps_sav{s}_{f}_{h}")
                        for ci, msz in enumerate((128, 68)):
                            e, c0 = es[h][ci]
                            nc.tensor.matmul(
                                ps_av[:],
                                st[s]["vs"][2 * f + ci][:, 65 * h : 65 * h + 65],
                                e[0:msz, c0 : c0 + 196],
                                start=(ci == 0),
                                stop=(ci == 1),
                            )
                        ps_avs.append(ps_av)
                    r = spool.tile([1, 392], cdt, tag="r", bufs=4,
                                   name=f"r{s}_{f}_{hp}")
                    with nc.allow_low_precision(reason="1/softmax-denom in bf16"):
                        for hi in range(2):
                            nc.vector.reciprocal(
                                r[0:1, 196 * hi : 196 * hi + 196],
                                ps_avs[hi][64:65, :],
                            )
                    ps_b = ppool.tile([64, 392], F32, tag="mm", bufs=2,
                                      name=f"ps_sb{s}_{f}_{hp}")
                    nc.tensor.matmul(ps_b[:], ones_bc[:], r[:], start=True, stop=True)
                    # HW allows only one PSUM operand per DVE op: stage the
                    # broadcast in SBUF (alternate engines for balance)
                    rb = spool.tile([64, 392], cdt, tag="rb", bufs=4,
                                    name=f"rb{s}_{f}_{hp}")
                    if (f + hp) % 2 == 0:
                        nc.scalar.copy(rb[:], ps_b[:])
                    else:
                        nc.vector.tensor_copy(rb[:], ps_b[:])
                    at = st[s]["attnT"][hp]
                    nc.vector.tensor_mul(
                        at[0:64, fo : fo + 196], ps_avs[0][0:64, :], rb[:, 0:196]
                    )
                    nc.vector.tensor_mul(
                        at[64:128, fo : fo + 196], ps_avs[1][0:64, :], rb[:, 196:392]
                    )

                return s2

            # ---- temporal scoring group (window w, heads hs) ---------------
            def tp_score(s, w, hs, ex):
                wo = 112 * w
                nh = len(hs)
                ems = ex["em"].setdefault(w, {})

                def s1():
                    ps_st = ppool.tile([112, 112 * nh], F32, tag="st", bufs=2,
                                       name=f"ps_tst{s}_{w}_{hs[0]}")
                    for idx, h in enumerate(hs):
                        pb = 64 * (h % 2)
                        qtile = st[s]["qkvt"][h // 2]
                        ktile = st[s]["qkvt"][6 + h // 2]
                        nc.tensor.matmul(
                            ps_st[:, 112 * idx : 112 * (idx + 1)],
                            ktile[pb : pb + 64, wo : wo + 112],
                            qtile[pb : pb + 64, wo : wo + 112],
                            start=True,
                            stop=True,
                        )
                    e = spool.tile([112, 112 * nh], cdt, tag="e", bufs=16,
                                   name=f"et{s}_{w}_{hs[0]}")
                    nc.scalar.activation(
                        e[:], ps_st[:], AF.Exp, bias=0.0, scale=SCALE,
                    )
                    em = spool.tile([112, 112 * nh], cdt, tag="em", bufs=8,
                                    name=f"emt{s}_{w}_{hs[0]}")
                    nc.gpsimd.tensor_mul(em[:], e[:], mask2_t[:, 0 : 112 * nh])
                    for idx, h in enumerate(hs):
                        ems[h] = (em, 112 * idx)
                return s1

            # ---- temporal normalize unit (window w, attnT pair 3+hp) -------
            def tp_norm(s, w, hp, ex):
                wo = 112 * w

                def s2():
                    ems = ex["em"][w]
                    ps_av = ppool.tile([65, 224], F32, tag="av", bufs=4,
                                       name=f"ps_tav{s}_{w}_{hp}")
                    for hi in range(2):
                        h = 6 + 2 * hp + hi
                        em, c0 = ems[h]
                        nc.tensor.matmul(
                            ps_av[:, 112 * hi : 112 * (hi + 1)],
                            st[s]["vt"][w][:, 65 * (h - 6) : 65 * (h - 6) + 65],
                            em[:, c0 : c0 + 112],
                            start=True,
                            stop=True,
                        )
                    r = spool.tile([1, 224], cdt, tag="rt", bufs=4,
                                   name=f"rt{s}_{w}_{hp}")
                    with nc.allow_low_precision(reason="1/softmax-denom in bf16"):
                        nc.vector.reciprocal(r[:], ps_av[64:65, :])
                    ps_b = ppool.tile([64, 224], F32, tag="mm", bufs=2,
                                      name=f"ps_tb{s}_{w}_{hp}")
                    nc.tensor.matmul(ps_b[:], ones_bc[:], r[:], start=True, stop=True)
                    rb = spool.tile([64, 224], cdt, tag="rbt", bufs=4,
                                    name=f"rbt{s}_{w}_{hp}")
                    if (w + hp) % 2 == 0:
                        nc.scalar.copy(rb[:], ps_b[:])
                    else:
                        nc.vector.tensor_copy(rb[:], ps_b[:])
                    at = st[s]["attnT"][3 + hp]
                    nc.vector.tensor_mul(
                        at[0:64, wo : wo + 112], ps_av[0:64, 0:112], rb[:, 0:112]
                    )
                    nc.vector.tensor_mul(
                        at[64:128, wo : wo + 112], ps_av[0:64, 112:224], rb[:, 112:224]
                    )

                return s2

            def out_group(s, ec, j):
                so = SB * s

                def emit():
                    ps = ppool.tile([128, 392], F32, tag="mm", bufs=2,
                                    name=f"ps_o{s}_{ec}_{j}")
                    for dc in range(6):
                        nc.tensor.matmul(
                            ps[:],
                            wp[dc][:, 128 * ec : 128 * (ec + 1)],
                            st[s]["attnT"][dc][:, 392 * j : 392 * (j + 1)],
                            start=(dc == 0),
                            stop=(dc == 5),
                        )
                    ot = spool.tile([128, 392], F32, tag="ot", bufs=4,
                                    name=f"ot{s}_{ec}_{j}")
                    nc.scalar.activation(
                        ot[:], ps[:], AF.Identity,
                        bias=bias_t[:, ec : ec + 1], scale=1.0,
                    )
                    nc.sync.dma_start(
                        out_d.ap()[
                            128 * ec : 128 * (ec + 1),
                            so + 392 * j : so + 392 * (j + 1),
                        ],
                        ot[:],
                    )
                return emit

            def proj_quanta(s):
                q = [qk_group(s, ti, j) for ti in range(12) for j in range(2)]
                q += [v_group(s, "s", i) for i in range(2 * FPSB)]
                q += [v_group(s, "t", w) for w in range(WPSB)]
                return q

            def sp_quanta(s, f, ex):
                # per-frame: 4 same-parity scoring groups + 3 normalize units
                return [
                    sp_score(s, f, [0, 2], ex),
                    sp_score(s, f, [1, 3], ex),
                    sp_norm(s, f, 0, ex),
                    sp_score(s, f, [4], ex),
                    sp_norm(s, f, 1, ex),
                    sp_score(s, f, [5], ex),
                    sp_norm(s, f, 2, ex),
                ]

            def tp_quanta(s, w, ex):
                return [
                    tp_score(s, w, [6, 8], ex),
                    tp_score(s, w, [7, 9], ex),
                    tp_norm(s, w, 0, ex),
                    tp_score(s, w, [10], ex),
                    tp_norm(s, w, 1, ex),
                    tp_score(s, w, [11], ex),
                    tp_norm(s, w, 2, ex),
                ]

            def attn_quanta_lists(s):
                ex = {"e": {}, "em": {}}
                sp = [q for f in range(FPSB) for q in sp_quanta(s, f, ex)]
                tp = [q for w in range(WPSB) for q in tp_quanta(s, w, ex)]
                return sp, tp

            import contextlib

            rep_ctx = (
                tc.For_i(0, reps, 1, hint_engines=(
                    mybir.EngineType.PE, mybir.EngineType.Activation,
                    mybir.EngineType.DVE, mybir.EngineType.SP,
                    mybir.EngineType.Pool))
                if reps > 1 else contextlib.nullcontext()
            )
            with rep_ctx:
                load_x(0)
                for q in proj_quanta(0):
                    q()
                last = NSB - 1
                for s in range(NSB):
                    sp, tp = attn_quanta_lists(s)
                    if s < last:
                        # PE-dense companions: out-proj of s-1 + projections
                        # of s+1 (x loads for s+1 emitted first for lead time)
                        load_x(s + 1)
                        b = []
                        if s >= 1:
                            b += [out_group(s - 1, ec, j)
                                  for ec in range(6) for j in range(2)]
                        b += proj_quanta(s + 1)
                        a = _interleave(sp, tp)
                        for q in _interleave(a, b):
                            q()
                    else:
                        # final superblock: first-half-token units weave with
                        # out-proj(s-1); second half weaves with the j=0
                        # out-proj groups of s (which only read cols 0:392)
                        nf, nw = 2 * 7, 4 * 7  # frames 0-1, windows 0-3
                        a1 = _interleave(sp[:nf], tp[:nw])
                        a2 = _interleave(sp[nf:], tp[nw:])
                        b1 = [out_group(s - 1, ec, j)
                              for ec in range(6) for j in range(2)]
                        for q in _interleave(a1, b1):
                            q()
                        b2 = [out_group(s, ec, 0) for ec in range(6)]
                        for q in _interleave(a2, b2):
                            q()
                        for q in [out_group(s, ec, 1) for ec in range(6)]:
                            q()

    nc.compile()
    return nc


def _get_nc(compute: str):
    if compute not in _CACHE:
        _CACHE[compute] = _build(compute)
    return _CACHE[compute]


def _np_dtype(compute: str):
    if compute == "f32":
        return np.float32
    import ml_dtypes

    return ml_dtypes.bfloat16


def kernel(x, w_qkv, w_proj, b_proj):
    nc = _get_nc(COMPUTE)
    dt = _np_dtype(COMPUTE)

    x = np.asarray(x, dtype=np.float32).reshape(B, N, D)
    xT = np.ascontiguousarray(x.transpose(0, 2, 1)).astype(dt)  # (B, D, N)
    wqkvT = np.ascontiguousarray(np.asarray(w_qkv, np.float32).T).astype(dt)
    wprojT = np.ascontiguousarray(np.asarray(w_proj, np.float32).T).astype(dt)
    bias = np.asarray(b_proj, np.float32).reshape(D, 1)

    mask = np.zeros((112, 112), np.float32)
    for g in range(7):
        mask[16 * g : 16 * (g + 1), 16 * g : 16 * (g + 1)] = 1.0
    mask = mask.astype(dt)

    in_maps = [
        {"xt": xT[b], "wqkvT": wqkvT, "wprojT": wprojT, "bias": bias, "mask": mask}
        for b in range(B)
    ]
    res = run_bass_kernel_spmd(nc, in_maps, core_ids=list(range(B)))
    out = np.stack([r["outT"].T for r in res.results])  # (B, N, D)
    return np.ascontiguousarray(out.reshape(B, F, P, D)).astype(np.float32)


if __name__ == "__main__":
    rng = np.random.default_rng(0)
    x = rng.standard_normal((B, F, P, D), dtype=np.float32)
    w_qkv = rng.standard_normal((E3, D), dtype=np.float32) * D**-0.5
    w_proj = rng.standard_normal((D, D), dtype=np.float32) * D**-0.5
    b_proj = np.zeros(D, np.float32)
    out = kernel(x=x, w_qkv=w_qkv, w_proj=w_proj, b_proj=b_proj)
    print(out.shape, out.dtype)


# revision 10
# speedup vs baseline: 1.0615x; 1.0615x over previous
"""Trainium2 Bass kernel for factorized space-time attention.

Computation (per batch b of 8, one NeuronCore each):
  qkv = x @ w_qkv.T                      (3136, 2304)
  heads 0-5:  spatial attention over 196 patches within each of 16 frames
  heads 6-11: temporal attention over groups of 16 consecutive tokens
  out = concat(head outputs) @ w_proj.T + b_proj

Strategy (data-parallel over batch, 8 cores):
  - bf16 matmul inputs (1 cycle/row on PE vs 4 for fp32), fp32 PSUM accum.
  - software-pipelined emission: attention of superblock s is interleaved
    instruction-by-instruction with the QKV/V projection of superblock s+1
    and the output projection of s-1, so the in-order PE queue always has
    dense matmul work while attention chains (exp -> AV -> recip ->
    broadcast -> mul) wait on ACT/DVE.
  - score/exp tiles batch SAME-PARITY head pairs ((h0,h2), (h1,h3), ...)
    per frame/window. HW constraint found empirically: matmuls writing the
    same PSUM bank must share the stationary partition quadrant
    (tile_position row); mixed-quadrant writers drain concurrently from
    independent PE sub-arrays and collide fatally in the bank.
  - softmax denominators come for free as row 64 of the AV matmul via a
    ones-column appended to V; 1/denom is broadcast across partitions with
    a [1,64]-ones stationary matmul, staged to SBUF (one PSUM operand max
    per DVE op), then two DVE multiplies write attnT rows 0-63 / 64-127
    directly (DVE partition offsets are legal at 32-granularity - no
    SBUF->SBUF shift DMAs).
  - temporal block-diagonal mask multiply runs on the otherwise-idle
    GpSimd engine (base tensor op; the extended partition_broadcast DKL
    instruction measured ~10x slower than modeled on HW - avoided).
"""

import sys

if "/opt/trn_rl_repo" not in sys.path:
    sys.path.append("/opt/trn_rl_repo")

import numpy as np

import concourse.bass as bass  # noqa: F401
import concourse.mybir as mybir
import concourse.tile as tile
from concourse import bacc
from concourse.bass_utils import run_bass_kernel_spmd

F32 = mybir.dt.float32
BF16 = mybir.dt.bfloat16
AF = mybir.ActivationFunctionType

# problem dims
B = 8
F = 16
P = 196
D = 768
NH = 12
HD = 64
N = F * P  # 3136
E3 = 3 * D  # 2304
SB = 784  # superblock = lcm(196, 16) tokens
NSB = N // SB  # 4
FPSB = SB // P  # 4 frames per superblock
WPSB = SB // 112  # 7 temporal windows per superblock
SCALE = HD ** -0.5

COMPUTE = "bf16"

_CACHE = {}


def _interleave(a, b):
    """Proportionally interleave two lists of thunks."""
    out = []
    la, lb = len(a), len(b)
    if la == 0:
        return list(b)
    if lb == 0:
        return list(a)
    ia = ib = 0
    tot = la + lb
    for k in range(tot):
        # emit from whichever stream is behind its proportional pace
        if ia * lb <= ib * la and ia < la:
            out.append(a[ia]); ia += 1
        elif ib < lb:
            out.append(b[ib]); ib += 1
        else:
            out.append(a[ia]); ia += 1
    return out


def _build(compute: str, reps: int = 1):
    cdt = BF16 if compute == "bf16" else F32

    nc = bacc.Bacc("TRN2", target_bir_lowering=False, debug=False, num_devices=B)

    xt_d = nc.dram_tensor("xt", (D, N), cdt, kind="ExternalInput")
    wqkv_d = nc.dram_tensor("wqkvT", (D, E3), cdt, kind="ExternalInput")
    wproj_d = nc.dram_tensor("wprojT", (D, D), cdt, kind="ExternalInput")
    bias_d = nc.dram_tensor("bias", (D, 1), F32, kind="ExternalInput")
    mask_d = nc.dram_tensor("mask", (112, 112), cdt, kind="ExternalInput")
    out_d = nc.dram_tensor("outT", (D, N), F32, kind="ExternalOutput")

    with tile.TileContext(nc) as tc:
        with (
            tc.tile_pool(name="const", bufs=1) as cpool,
            tc.tile_pool(name="work", bufs=1) as wpool,
            tc.tile_pool(name="small", bufs=4) as spool,
            tc.tile_pool(name="psum", bufs=2, space="PSUM") as ppool,
        ):
            # ---- constants -------------------------------------------------
            wq = []
            for dc in range(6):
                t = cpool.tile([128, E3], cdt, tag=f"wq{dc}", name=f"wq{dc}")
                nc.sync.dma_start(t[:], wqkv_d.ap()[128 * dc : 128 * (dc + 1), :])
                wq.append(t)
            wp = []
            for dc in range(6):
                t = cpool.tile([128, D], cdt, tag=f"wp{dc}", name=f"wp{dc}")
                nc.sync.dma_start(t[:], wproj_d.ap()[128 * dc : 128 * (dc + 1), :])
                wp.append(t)
            bias_t = cpool.tile([128, 6], F32, tag="bias", name="bias_t")
            nc.sync.dma_start(
                bias_t[:], bias_d.ap().rearrange("(e p) one -> p (e one)", p=128)
            )
            mask2_t = cpool.tile([112, 224], cdt, tag="mask", name="mask2_t")
            nc.sync.dma_start(mask2_t[:, 0:112], mask_d.ap())
            nc.sync.dma_start(mask2_t[:, 112:224], mask_d.ap())
            zeros_col = cpool.tile([128, 1], F32, tag="zeros_c", name="zeros_col")
            nc.gpsimd.memset(zeros_col[:], 0.0)
            # stationary [1, 64] of ones for the 1/sum partition-broadcast
            ones_bc = cpool.tile([1, 64], cdt, tag="ones_bc", name="ones_bc")
            nc.gpsimd.memset(ones_bc[:], 1.0)

            # ---- per-superblock state (allocated lazily inside quanta) ----
            st = [dict() for _ in range(NSB)]

            def load_x(s):
                so = SB * s
                xts = []
                for dc in range(6):
                    t = wpool.tile([128, SB], cdt, tag=f"xts{dc}", bufs=2,
                                   name=f"xts{dc}_{s}")
                    nc.sync.dma_start(
                        t[:], xt_d.ap()[128 * dc : 128 * (dc + 1), so : so + SB]
                    )
                    xts.append(t)
                st[s]["xts"] = xts
                st[s]["qkvt"] = [None] * 12
                st[s]["vs"] = [None] * (2 * FPSB)
                st[s]["vt"] = [None] * WPSB
                st[s]["attnT"] = [
                    wpool.tile([128, SB], cdt, tag=f"attnT{i}", bufs=2,
                               name=f"attnT{i}_{s}")
                    for i in range(6)
                ]

            def qk_group(s, ti, j):
                def emit():
                    xts = st[s]["xts"]
                    if st[s]["qkvt"][ti] is None:
                        st[s]["qkvt"][ti] = wpool.tile(
                            [128, SB], cdt, tag=f"qkvt{ti}", bufs=2,
                            name=f"qkvt{ti}_{s}")
                    qt = st[s]["qkvt"][ti]
                    ps = ppool.tile([128, 392], F32, tag="mm", bufs=2,
                                    name=f"ps_qk{s}_{ti}_{j}")
                    for dc in range(6):
                        nc.tensor.matmul(
                            ps[:],
                            wq[dc][:, 128 * ti : 128 * (ti + 1)],
                            xts[dc][:, 392 * j : 392 * (j + 1)],
                            start=(dc == 0),
                            stop=(dc == 5),
                        )
                    # balance PSUM->SBUF evacuation across DVE / ACT
                    if (ti + j) % 2 == 0:
                        nc.vector.tensor_copy(qt[:, 392 * j : 392 * (j + 1)], ps[:])
                    else:
                        nc.scalar.copy(qt[:, 392 * j : 392 * (j + 1)], ps[:])
                return emit

            def v_group(s, kind, idx):
                # kind "s": spatial per-frame chunks (idx = 2*f + ci)
                # kind "t": temporal 112-token windows (idx = w)
                def emit():
                    xts = st[s]["xts"]
                    if kind == "s":
                        f, ci = idx // 2, idx % 2
                        m0, msz = (0, 128) if ci == 0 else (128, 68)
                        tok0, wcol0 = 196 * f + m0, 1536
                        vname = f"vs{f}_{ci}_{s}"
                    else:
                        msz, tok0, wcol0 = 112, 112 * idx, 1920
                        vname = f"vt{idx}_{s}"
                    vt_ = wpool.tile([msz, 390], cdt, tag=f"v{kind}{idx}", bufs=2,
                                     name=vname)
                    ps = ppool.tile([msz, 384], F32, tag="mm", bufs=2,
                                    name=f"ps_v{kind}{s}_{idx}")
                    for dc in range(6):
                        nc.tensor.matmul(
                            ps[:],
                            xts[dc][:, tok0 : tok0 + msz],
                            wq[dc][:, wcol0 : wcol0 + 384],
                            start=(dc == 0),
                            stop=(dc == 5),
                        )
                    nc.scalar.copy(
                        vt_.rearrange("p (h c) -> p h c", c=65)[:, :, 0:64],
                        ps.rearrange("p (h c) -> p h c", c=64),
                    )
                    nc.gpsimd.memset(
                        vt_.rearrange("p (h c) -> p h c", c=65)[:, :, 64:65], 1.0
                    )
                    if kind == "s":
                        st[s]["vs"][idx] = vt_
                    else:
                        st[s]["vt"][idx] = vt_
                return emit

            # HW rule (found empirically): matmuls writing the same PSUM bank
            # must share the stationary partition quadrant (tile_position
            # row). Mixed-quadrant writers drain concurrently from
            # independent PE sub-arrays and collide fatally. So score tiles
            # batch SAME-PARITY heads: (h0,h2), (h1,h3), and singles.

            # ---- spatial scoring group (frame f, heads hs, parity rows) ----
            def sp_score(s, f, hs, ex):
                # hs: list of same-parity global spatial heads (1 or 2)
                fo = 196 * f
                nh = len(hs)
                es = ex["e"].setdefault(f, {})

                def s1():
                    for ci, (m0, msz) in enumerate(((0, 128), (128, 68))):
                        ps_st = ppool.tile([msz, 196 * nh], F32, tag="st", bufs=3,
                                           name=f"ps_st{s}_{f}_{hs[0]}_{ci}")
                        for idx, h in enumerate(hs):
                            pb = 64 * (h % 2)
                            qtile = st[s]["qkvt"][h // 2]
                            ktile = st[s]["qkvt"][6 + h // 2]
                            nc.tensor.matmul(
                                ps_st[:, 196 * idx : 196 * (idx + 1)],
                                ktile[pb : pb + 64, fo + m0 : fo + m0 + msz],
                                qtile[pb : pb + 64, fo : fo + 196],
                                start=True,
                                stop=True,
                            )
                        e = spool.tile([msz, 196 * nh], cdt, tag="e", bufs=16,
                                       name=f"e{s}_{f}_{hs[0]}_{ci}")
                        nc.scalar.activation(
                            e[:], ps_st[:], AF.Exp, bias=0.0, scale=SCALE,
                        )
                        for idx, h in enumerate(hs):
                            es.setdefault(h, [None, None])
                            es[h][ci] = (e, 196 * idx)
                return s1

            # ---- spatial normalize unit (frame f, attnT pair hp) -----------
            def sp_norm(s, f, hp, ex):
                fo = 196 * f

                def s2():
                    es = ex["e"][f]
                    ps_avs = []
                    for hi in range(2):
                        h = 2 * hp + hi
                        ps_av = ppool.tile([65, 196], F32, tag="av", bufs=3,
                                           name=f"ps_sav{s}_{f}_{h}")
                        for ci, msz in enumerate((128, 68)):
                            e, c0 = es[h][ci]
                            nc.tensor.matmul(
                                ps_av[:],
                                st[s]["vs"][2 * f + ci][:, 65 * h : 65 * h + 65],
                                e[0:msz, c0 : c0 + 196],
                                start=(ci == 0),
                                stop=(ci == 1),
                            )
                        ps_avs.append(ps_av)
                    r = spool.tile([1, 392], cdt, tag="r", bufs=4,
                                   name=f"r{s}_{f}_{hp}")
                    with nc.allow_low_precision(reason="1/softmax-denom in bf16"):
                        for hi in range(2):
                            nc.vector.reciprocal(
                                r[0:1, 196 * hi : 196 * hi + 196],
                                ps_avs[hi][64:65, :],
                            )
                    ps_b = ppool.tile([64, 392], F32, tag="mm", bufs=2,
                                      name=f"ps_sb{s}_{f}_{hp}")
                    nc.tensor.matmul(ps_b[:], ones_bc[:], r[:], start=True, stop=True)
                    # HW allows only one PSUM operand per DVE op: stage the
                    # broadcast in SBUF (alternate engines for balance)
                    rb = spool.tile([64, 392], cdt, tag="rb", bufs=4,
                                    name=f"rb{s}_{f}_{hp}")
                    if (f + hp) % 2 == 0:
                        nc.scalar.copy(rb[:], ps_b[:])
                    else:
                        nc.vector.tensor_copy(rb[:], ps_b[:])
                    at = st[s]["attnT"][hp]
                    nc.vector.tensor_mul(
                        at[0:64, fo : fo + 196], ps_avs[0][0:64, :], rb[:, 0:196]
                    )
                    nc.vector.tensor_mul(
                        at[64:128, fo : fo + 196], ps_avs[1][0:64, :], rb[:, 196:392]
                    )

                return s2

            # ---- temporal scoring group (window w, heads hs) ---------------
            def tp_score(s, w, hs, ex):
                wo = 112 * w
                nh = len(hs)
                ems = ex["em"].setdefault(w, {})

                def s1():
                    ps_st = ppool.tile([112, 112 * nh], F32, tag="st", bufs=3,
                                       name=f"ps_tst{s}_{w}_{hs[0]}")
                    for idx, h in enumerate(hs):
                        pb = 64 * (h % 2)
                        qtile = st[s]["qkvt"][h // 2]
                        ktile = st[s]["qkvt"][6 + h // 2]
                        nc.tensor.matmul(
                            ps_st[:, 112 * idx : 112 * (idx + 1)],
                            ktile[pb : pb + 64, wo : wo + 112],
                            qtile[pb : pb + 64, wo : wo + 112],
                            start=True,
                            stop=True,
                        )
                    e = spool.tile([112, 112 * nh], cdt, tag="e", bufs=16,
                                   name=f"et{s}_{w}_{hs[0]}")
                    nc.scalar.activation(
                        e[:], ps_st[:], AF.Exp, bias=0.0, scale=SCALE,
                    )
                    em = spool.tile([112, 112 * nh], cdt, tag="em", bufs=8,
                                    name=f"emt{s}_{w}_{hs[0]}")
                    nc.gpsimd.tensor_mul(em[:], e[:], mask2_t[:, 0 : 112 * nh])
                    for idx, h in enumerate(hs):
                        ems[h] = (em, 112 * idx)
                return s1

            # ---- temporal normalize unit (window w, attnT pair 3+hp) -------
            def tp_norm(s, w, hp, ex):
                wo = 112 * w

                def s2():
                    ems = ex["em"][w]
                    ps_av = ppool.tile([65, 224], F32, tag="av", bufs=3,
                                       name=f"ps_tav{s}_{w}_{hp}")
                    for hi in range(2):
                        h = 6 + 2 * hp + hi
                        em, c0 = ems[h]
                        nc.tensor.matmul(
                            ps_av[:, 112 * hi : 112 * (hi + 1)],
                            st[s]["vt"][w][:, 65 * (h - 6) : 65 * (h - 6) + 65],
                            em[:, c0 : c0 + 112],
                            start=True,
                            stop=True,
                        )
                    r = spool.tile([1, 224], cdt, tag="rt", bufs=4,
                                   name=f"rt{s}_{w}_{hp}")
                    with nc.allow_low_precision(reason="1/softmax-denom in bf16"):
                        nc.vector.reciprocal(r[:], ps_av[64:65, :])
                    ps_b = ppool.tile([64, 224], F32, tag="mm", bufs=2,
                                      name=f"ps_tb{s}_{w}_{hp}")
                    nc.tensor.matmul(ps_b[:], ones_bc[:], r[:], start=True, stop=True)
                    rb = spool.tile([64, 224], cdt, tag="rbt", bufs=4,
                                    name=f"rbt{s}_{w}_{hp}")
                    if (w + hp) % 2 == 0:
                        nc.scalar.copy(rb[:], ps_b[:])
                    else:
                        nc.vector.tensor_copy(rb[:], ps_b[:])
                    at = st[s]["attnT"][3 + hp]
                    nc.vector.tensor_mul(
                        at[0:64, wo : wo + 112], ps_av[0:64, 0:112], rb[:, 0:112]
                    )
                    nc.vector.tensor_mul(
                        at[64:128, wo : wo + 112], ps_av[0:64, 112:224], rb[:, 112:224]
                    )

                return s2

            def out_group(s, ec, j):
                so = SB * s

                def emit():
                    ps = ppool.tile([128, 392], F32, tag="mm", bufs=2,
                                    name=f"ps_o{s}_{ec}_{j}")
                    for dc in range(6):
                        nc.tensor.matmul(
                            ps[:],
                            wp[dc][:, 128 * ec : 128 * (ec + 1)],
                            st[s]["attnT"][dc][:, 392 * j : 392 * (j + 1)],
                            start=(dc == 0),
                            stop=(dc == 5),
                        )
                    ot = spool.tile([128, 392], F32, tag="ot", bufs=4,
                                    name=f"ot{s}_{ec}_{j}")
                    nc.scalar.activation(
                        ot[:], ps[:], AF.Identity,
                        bias=bias_t[:, ec : ec + 1], scale=1.0,
                    )
                    nc.sync.dma_start(
                        out_d.ap()[
                            128 * ec : 128 * (ec + 1),
                            so + 392 * j : so + 392 * (j + 1),
                        ],
                        ot[:],
                    )
                return emit

            def proj_quanta(s):
                q = [qk_group(s, ti, j) for ti in range(12) for j in range(2)]
                q += [v_group(s, "s", i) for i in range(2 * FPSB)]
                q += [v_group(s, "t", w) for w in range(WPSB)]
                return q

            def sp_quanta(s, f, ex):
                # per-frame: 4 same-parity scoring groups + 3 normalize units
                return [
                    sp_score(s, f, [0, 2], ex),
                    sp_score(s, f, [1, 3], ex),
                    sp_norm(s, f, 0, ex),
                    sp_score(s, f, [4], ex),
                    sp_norm(s, f, 1, ex),
                    sp_score(s, f, [5], ex),
                    sp_norm(s, f, 2, ex),
                ]

            def tp_quanta(s, w, ex):
                return [
                    tp_score(s, w, [6, 8], ex),
                    tp_score(s, w, [7, 9], ex),
                    tp_norm(s, w, 0, ex),
                    tp_score(s, w, [10], ex),
                    tp_norm(s, w, 1, ex),
                    tp_score(s, w, [11], ex),
                    tp_norm(s, w, 2, ex),
                ]

            def attn_quanta_lists(s):
                ex = {"e": {}, "em": {}}
                sp = [q for f in range(FPSB) for q in sp_quanta(s, f, ex)]
                tp = [q for w in range(WPSB) for q in tp_quanta(s, w, ex)]
                return sp, tp

            import contextlib

            rep_ctx = (
                tc.For_i(0, reps, 1, hint_engines=(
                    mybir.EngineType.PE, mybir.EngineType.Activation,
                    mybir.EngineType.DVE, mybir.EngineType.SP,
                    mybir.EngineType.Pool))
                if reps > 1 else contextlib.nullcontext()
            )
            with rep_ctx:
                load_x(0)
                for q in proj_quanta(0):
                    q()
                last = NSB - 1
                for s in range(NSB):
                    sp, tp = attn_quanta_lists(s)
                    if s < last:
                        # PE-dense companions: out-proj of s-1 + projections
                        # of s+1 (x loads for s+1 emitted first for lead time)
                        load_x(s + 1)
                        b = []
                        if s >= 1:
                            b += [out_group(s - 1, ec, j)
                                  for ec in range(6) for j in range(2)]
                        b += proj_quanta(s + 1)
                        a = _interleave(sp, tp)
                        for q in _interleave(a, b):
                            q()
                    else:
                        # final superblock: first-half-token units weave with
                        # out-proj(s-1); second half weaves with the j=0
                        # out-proj groups of s (which only read cols 0:392)
                        nf, nw = 2 * 7, 4 * 7  # frames 0-1, windows 0-3
                        a1 = _interleave(sp[:nf], tp[:nw])
                        a2 = _interleave(sp[nf:], tp[nw:])
                        b1 = [out_group(s - 1, ec, j)
                              for ec in range(6) for j in range(2)]
                        for q in _interleave(a1, b1):
                            q()
                        b2 = [out_group(s, ec, 0) for ec in range(6)]
                        for q in _interleave(a2, b2):
                            q()
                        for q in [out_group(s, ec, 1) for ec in range(6)]:
                            q()

    nc.compile()
    return nc


def _get_nc(compute: str):
    if compute not in _CACHE:
        _CACHE[compute] = _build(compute)
    return _CACHE[compute]


def _np_dtype(compute: str):
    if compute == "f32":
        return np.float32
    import ml_dtypes

    return ml_dtypes.bfloat16


def kernel(x, w_qkv, w_proj, b_proj):
    nc = _get_nc(COMPUTE)
    dt = _np_dtype(COMPUTE)

    x = np.asarray(x, dtype=np.float32).reshape(B, N, D)
    xT = np.ascontiguousarray(x.transpose(0, 2, 1)).astype(dt)  # (B, D, N)
    wqkvT = np.ascontiguousarray(np.asarray(w_qkv, np.float32).T).astype(dt)
    wprojT = np.ascontiguousarray(np.asarray(w_proj, np.float32).T).astype(dt)
    bias = np.asarray(b_proj, np.float32).reshape(D, 1)

    mask = np.zeros((112, 112), np.float32)
    for g in range(7):
        mask[16 * g : 16 * (g + 1), 16 * g : 16 * (g + 1)] = 1.0
    mask = mask.astype(dt)

    in_maps = [
        {"xt": xT[b], "wqkvT": wqkvT, "wprojT": wprojT, "bias": bias, "mask": mask}
        for b in range(B)
    ]
    res = run_bass_kernel_spmd(nc, in_maps, core_ids=list(range(B)))
    out = np.stack([r["outT"].T for r in res.results])  # (B, N, D)
    return np.ascontiguousarray(out.reshape(B, F, P, D)).astype(np.float32)


if __name__ == "__main__":
    rng = np.random.default_rng(0)
    x = rng.standard_normal((B, F, P, D), dtype=np.float32)
    w_qkv = rng.standard_normal((E3, D), dtype=np.float32) * D**-0.5
    w_proj = rng.standard_normal((D, D), dtype=np.float32) * D**-0.5
    b_proj = np.zeros(D, np.float32)
    out = kernel(x=x, w_qkv=w_qkv, w_proj=w_proj, b_proj=b_proj)
    print(out.shape, out.dtype)
